# revision 9
# baseline (speedup 1.0000x reference)
"""Trainium2 Bass kernel for nn_CriticNetwork (LSTM T=3, D=18, H=64 + MLP 64->32->1).

Strategy: pure data parallel over 8 NeuronCores (65536 batch each).
Per core, batch is processed in 64 iterations of 1024 elements
(two 512-wide sub-tiles A/B occupying partition rows 0:64 / 64:128 of
every pointwise tile).

Layout: feature-rows on partitions, batch on the free dim.
Gate preacts are produced by single K-stacked bf16 matmuls
  rhs = [h (64) ; x_t (18) ; ones (2: bias hi/lo)]  -> K=84 (step1: K=20)
into PSUM chunks [i|f|o] (3 banks) and [g] (1 bank).
sigmoid/tanh_g run on ScalarE straight from PSUM (bf16 out),
tanh(c) is a custom 8-stage DVE op (odd deg-7 poly, |c|<=1.5),
products run on VectorE in bf16 (2x mode), c = ig+fc on GpSimd.
MLP + final dot use tile-positioned small matmuls.
"""
import os
import numpy as np
import ml_dtypes

import concourse.bacc as bacc
import concourse.bass as bass
import concourse.mybir as mybir
import concourse.tile as tile
from concourse import bass_utils

F32 = mybir.dt.float32
BF16 = mybir.dt.bfloat16
AF = mybir.ActivationFunctionType

NCORES = 8
W = 512                        # sub-tile width (psum bank)
ITERS = int(os.environ.get("K_ITERS", "64"))
BCORE = ITERS * 2 * W          # 65536 at full size
BATCH = BCORE * NCORES
BLK = 4                        # iters per DMA block
NBLK = ITERS // BLK

STATE_DIM, SEQ_LEN, HIDDEN, MLP_HIDDEN = 18, 3, 64, 32
KX = STATE_DIM + 2             # x rows + two ones rows (bias hi/lo)
KS = HIDDEN + KX               # 84: [h; x; ones]

# tanh deg-7 odd poly on [-1.5, 1.5], maxerr 5.1e-4
T7C = (0.9967175625159229, -0.3102624127429846,
       0.08661915494425512, -0.011767701262857437)

_tanh7c = None


def get_tanh7c():
    global _tanh7c
    if _tanh7c is not None:
        return _tanh7c
    import concourse.dve_ops as dve_ops
    from concourse.dve_spec import (Spec, Src0, C0, C1, C2, C3, sq, lower,
                                    _spill_c3_to_src1)
    from concourse.dve_uop import DveOpSpec

    name = "TANH7C_ANT"
    for op in dve_ops.OPS:
        if op.name == name:
            _tanh7c = op
            return op
    u = sq(Src0)
    body = _spill_c3_to_src1((((C3 * u + C2) * u + C1) * u + C0) * Src0)

    def _ref(in0, in1, s0, s1, imm2):
        uu = in0.astype(np.float32) ** 2
        c3 = np.asarray(in1, np.float32).reshape(in1.shape[0], 1)
        return ((((c3 * uu + imm2) * uu + s1) * uu) + s0) * in0

    spec = Spec(body=body, reference=_ref)
    if name not in dve_ops._SUB_OPCODE_FOR_NAME:
        dve_ops._SUB_OPCODE_FOR_NAME[name] = (
            max(dve_ops._SUB_OPCODE_FOR_NAME.values()) + 1)
    shas = {}
    for ver in ("v3", "v4"):
        try:
            s = DveOpSpec(name=name, opcode=dve_ops.get_dve_sub_opcode(name),
                          uops=lower(spec, ver=ver), rd1_en=True)
            shas[ver] = s.sha(ver)
        except Exception:
            pass
    op = dve_ops.DveOp(name, spec, subdim=False, uops_sha=shas)
    dve_ops.OPS.append(op)
    _tanh7c = op
    return op


# p_ifo free-dim slots (cols) per gate; g goes to its own bank
SLOT = {"i": 0, "f": W, "o": 2 * W}
# weight-matrix column base per gate in the packed wk tiles: [i f o g]
WCOL = {"i": 0, "f": 64, "o": 128, "g": 192}


def build_bass():
    t7 = get_tanh7c()
    nc = bacc.Bacc("TRN2", target_bir_lowering=False, debug=False)

    xt0_d = nc.dram_tensor("xt0", [KX, BCORE], BF16, kind="ExternalInput").ap()
    xt1_d = nc.dram_tensor("xt1", [KX, BCORE], BF16, kind="ExternalInput").ap()
    xt2_d = nc.dram_tensor("xt2", [KX, BCORE], BF16, kind="ExternalInput").ap()
    wk0_d = nc.dram_tensor("wk0", [KX, 256], BF16, kind="ExternalInput").ap()
    wks_d = nc.dram_tensor("wks", [KS, 256], BF16, kind="ExternalInput").ap()
    w1_d = nc.dram_tensor("w1r", [128, MLP_HIDDEN], BF16, kind="ExternalInput").ap()
    w2_d = nc.dram_tensor("w2r", [64, 1], BF16, kind="ExternalInput").ap()
    b1_d = nc.dram_tensor("b1r", [64, 1], F32, kind="ExternalInput").ap()
    b2_d = nc.dram_tensor("b2r", [128, 1], F32, kind="ExternalInput").ap()
    out_d = nc.dram_tensor("out", [2 * ITERS, W], F32, kind="ExternalOutput").ap()

    BW = BLK * 2 * W  # dma block width (4096)

    with tile.TileContext(nc) as tc:
        with tc.tile_pool(name="const", bufs=1) as cpool, \
             tc.tile_pool(name="xt0", bufs=2) as xt0p, \
             tc.tile_pool(name="s1", bufs=2) as s1p, \
             tc.tile_pool(name="s2", bufs=2) as s2p, \
             tc.tile_pool(name="sg", bufs=3) as sgp, \
             tc.tile_pool(name="tg", bufs=3) as tgp, \
             tc.tile_pool(name="work", bufs=3) as wkp, \
             tc.tile_pool(name="h3", bufs=2) as h3p, \
             tc.tile_pool(name="zr", bufs=2) as zrp, \
             tc.tile_pool(name="vout", bufs=2) as vop, \
             tc.tile_pool(name="pifo", bufs=1, space="PSUM") as pifop, \
             tc.tile_pool(name="pg", bufs=2, space="PSUM") as pgp, \
             tc.tile_pool(name="pz", bufs=2, space="PSUM") as pzp, \
             tc.tile_pool(name="pv", bufs=1, space="PSUM") as pvp:

            wk0 = cpool.tile([KX, 256], BF16)
            nc.sync.dma_start(wk0[:], wk0_d[:])
            wks = cpool.tile([KS, 256], BF16)
            nc.sync.dma_start(wks[:], wks_d[:])
            w1r = cpool.tile([128, MLP_HIDDEN], BF16)
            nc.sync.dma_start(w1r[:], w1_d[:])
            w2r = cpool.tile([64, 1], BF16)
            nc.sync.dma_start(w2r[:], w2_d[:])
            b1r = cpool.tile([64, 1], F32)
            nc.sync.dma_start(b1r[:], b1_d[:])
            b2t = cpool.tile([128, 1], F32)
            nc.sync.dma_start(b2t[:], b2_d[:])
            c3t = cpool.tile([128, 1], F32)
            nc.vector.memset(c3t[:], T7C[3])

            def gate_mms(wk, kk, rhs_blk, cA, p_ifo, p_g, gates):
                """Emit A/B col-paired matmuls for `gates` of one step.
                rhs_blk: block tile whose rows 0:kk are the stacked rhs;
                cA: column offset of sub-tile A inside the block."""
                for gn in gates:
                    lhs = wk[0:kk, WCOL[gn]:WCOL[gn] + 64]
                    if gn == "g":
                        oA, oB = p_g[0:64, :], p_g[64:128, :]
                    else:
                        s = SLOT[gn]
                        oA, oB = p_ifo[0:64, s:s + W], p_ifo[64:128, s:s + W]
                    nc.tensor.matmul(oA, lhs, rhs_blk[0:kk, cA:cA + W],
                                     start=True, stop=True, tile_position=(0, 0),
                                     skip_group_check=True)
                    nc.tensor.matmul(oB, lhs, rhs_blk[0:kk, cA + W:cA + 2 * W],
                                     start=True, stop=True, tile_position=(0, 64),
                                     skip_group_check=True)

            vb = None
            for blk in range(NBLK):
                xt0b = xt0p.tile([KX, BW], BF16)
                nc.sync.dma_start(xt0b[:], xt0_d[:, blk * BW:(blk + 1) * BW])
                s1b = s1p.tile([KS, BW], BF16)
                nc.sync.dma_start(s1b[HIDDEN:KS, :], xt1_d[:, blk * BW:(blk + 1) * BW])
                s2b = s2p.tile([KS, BW], BF16)
                nc.sync.dma_start(s2b[HIDDEN:KS, :], xt2_d[:, blk * BW:(blk + 1) * BW])

                for j in range(BLK):
                    it = blk * BLK + j
                    cA = j * 2 * W

                    # ---------------- step 1 (h0 = 0): gates i, o, g ----------
                    p_ifo = pifop.tile([128, 3 * W], F32, tag="pifo")
                    p_g = pgp.tile([128, W], F32, tag="pg")
                    gate_mms(wk0, KX, xt0b, cA, p_ifo, p_g, ("i", "o", "g"))

                    sg1 = sgp.tile([128, 3 * W], BF16, tag="sg")
                    nc.scalar.activation(sg1[:, 0:W], p_ifo[:, 0:W], AF.Sigmoid)
                    nc.scalar.activation(sg1[:, 2 * W:3 * W], p_ifo[:, 2 * W:3 * W],
                                         AF.Sigmoid)
                    tg1 = tgp.tile([128, W], BF16, tag="tg")
                    nc.scalar.activation(tg1[:], p_g[:], AF.Tanh)

                    c1 = wkp.tile([128, W], BF16, tag="c")
                    nc.vector.tensor_mul(c1[:], sg1[:, 0:W], tg1[:])
                    th1 = wkp.tile([128, W], BF16, tag="th")
                    nc.vector._custom_dve(t7, out=th1[:], in0=c1[:], in1=c3t[:],
                                          s0=T7C[0], s1=T7C[1], imm2=T7C[2])
                    # h1 -> s1 block rows 0:64 (A and B halves)
                    nc.vector.tensor_mul(s1b[0:HIDDEN, cA:cA + W],
                                         sg1[0:64, 2 * W:3 * W], th1[0:64, :])
                    nc.vector.tensor_mul(s1b[0:HIDDEN, cA + W:cA + 2 * W],
                                         sg1[64:128, 2 * W:3 * W], th1[64:128, :])

                    # ---------------- step 2: gates i, f, o, g ----------------
                    p_ifo2 = pifop.tile([128, 3 * W], F32, tag="pifo")
                    p_g2 = pgp.tile([128, W], F32, tag="pg")
                    gate_mms(wks, KS, s1b, cA, p_ifo2, p_g2, ("i", "f", "o", "g"))

                    sg2 = sgp.tile([128, 3 * W], BF16, tag="sg")
                    nc.scalar.activation(sg2[:], p_ifo2[:], AF.Sigmoid)
                    tg2 = tgp.tile([128, W], BF16, tag="tg")
                    nc.scalar.activation(tg2[:], p_g2[:], AF.Tanh)

                    ig2 = wkp.tile([128, W], BF16, tag="ig")
                    nc.vector.tensor_mul(ig2[:], sg2[:, 0:W], tg2[:])
                    fc2 = wkp.tile([128, W], BF16, tag="fc")
                    nc.vector.tensor_mul(fc2[:], sg2[:, W:2 * W], c1[:])
                    c2 = wkp.tile([128, W], BF16, tag="c")
                    nc.gpsimd.tensor_add(c2[:], ig2[:], fc2[:])
                    th2 = wkp.tile([128, W], BF16, tag="th")
                    nc.vector._custom_dve(t7, out=th2[:], in0=c2[:], in1=c3t[:],
                                          s0=T7C[0], s1=T7C[1], imm2=T7C[2])
                    nc.vector.tensor_mul(s2b[0:HIDDEN, cA:cA + W],
                                         sg2[0:64, 2 * W:3 * W], th2[0:64, :])
                    nc.vector.tensor_mul(s2b[0:HIDDEN, cA + W:cA + 2 * W],
                                         sg2[64:128, 2 * W:3 * W], th2[64:128, :])

                    # ---------------- step 3 ----------------------------------
                    p_ifo3 = pifop.tile([128, 3 * W], F32, tag="pifo")
                    p_g3 = pgp.tile([128, W], F32, tag="pg")
                    gate_mms(wks, KS, s2b, cA, p_ifo3, p_g3, ("i", "f", "o", "g"))

                    sg3 = sgp.tile([128, 3 * W], BF16, tag="sg")
                    nc.scalar.activation(sg3[:], p_ifo3[:], AF.Sigmoid)
                    tg3 = tgp.tile([128, W], BF16, tag="tg")
                    nc.scalar.activation(tg3[:], p_g3[:], AF.Tanh)

                    ig3 = wkp.tile([128, W], BF16, tag="ig")
                    nc.vector.tensor_mul(ig3[:], sg3[:, 0:W], tg3[:])
                    fc3 = wkp.tile([128, W], BF16, tag="fc")
                    nc.vector.tensor_mul(fc3[:], sg3[:, W:2 * W], c2[:])
                    c3_ = wkp.tile([128, W], BF16, tag="c")
                    nc.gpsimd.tensor_add(c3_[:], ig3[:], fc3[:])
                    th3 = wkp.tile([128, W], BF16, tag="th")
                    nc.vector._custom_dve(t7, out=th3[:], in0=c3_[:], in1=c3t[:],
                                          s0=T7C[0], s1=T7C[1], imm2=T7C[2])
                    h3 = h3p.tile([128, W], BF16, tag="h3")
                    nc.vector.tensor_mul(h3[:], sg3[:, 2 * W:3 * W], th3[:])

                    # ---------------- MLP + value ------------------------------
                    zp = pzp.tile([64, W], F32, tag="zp")
                    nc.tensor.matmul(zp[0:32, :], w1r[0:64, :], h3[0:64, :],
                                     start=True, stop=True, tile_position=(0, 0),
                                     skip_group_check=True)
                    nc.tensor.matmul(zp[32:64, :], w1r[64:128, :], h3[64:128, :],
                                     start=True, stop=True, tile_position=(64, 32),
                                     skip_group_check=True)
                    zr = zrp.tile([64, W], BF16, tag="zr")
                    nc.scalar.activation(zr[:], zp[:], AF.Relu, bias=b1r[:])

                    if it % 2 == 0:
                        vb = pvp.tile([97, W], F32, tag="vb")
                    r = (it % 2) * 64
                    nc.tensor.matmul(vb[r:r + 1, :], w2r[0:32, :], zr[0:32, :],
                                     start=True, stop=True, tile_position=(0, r),
                                     skip_group_check=True)
                    nc.tensor.matmul(vb[r + 32:r + 33, :], w2r[32:64, :],
                                     zr[32:64, :], start=True, stop=True,
                                     tile_position=(32, r + 32),
                                     skip_group_check=True)

                    if it % 2 == 1:
                        vo = vop.tile([97, W], F32, tag="vo")
                        nc.scalar.activation(vo[:], vb[:], AF.Identity,
                                             bias=b2t[0:97, :])
                        e = it - 1
                        for s, row in ((2 * e, 0), (2 * e + 1, 32),
                                       (2 * e + 2, 64), (2 * e + 3, 96)):
                            nc.sync.dma_start(out_d[s:s + 1, :], vo[row:row + 1, :])

    nc.compile()
    return nc


def _host_prep(state_seq, W_ih, W_hh, b_ih, b_hh, W1, b1, W2, b2):
    """Build per-core input maps (host-side layout prep only)."""
    bf = ml_dtypes.bfloat16
    B = state_seq.shape[0]
    b = (b_ih.astype(np.float64) + b_hh.astype(np.float64))  # [256]

    # gate reorder [i f o g] columns
    perm = np.concatenate([np.arange(0, 64), np.arange(64, 128),
                           np.arange(192, 256), np.arange(128, 192)])
    # packed weight cols: wk[:, c] for output gate-row perm[c]
    Wih_p = W_ih[perm, :].astype(np.float64)   # [256, 18]
    Whh_p = W_hh[perm, :].astype(np.float64)   # [256, 64]
    b_p = b[perm]                               # [256]

    b_hi = b_p.astype(bf).astype(np.float64)
    b_lo = (b_p - b_hi).astype(bf).astype(np.float64)

    wk0 = np.zeros((KX, 256), np.float64)
    wk0[0:STATE_DIM, :] = Wih_p.T
    wk0[STATE_DIM, :] = b_hi
    wk0[STATE_DIM + 1, :] = b_lo
    wks = np.zeros((KS, 256), np.float64)
    wks[0:HIDDEN, :] = Whh_p.T
    wks[HIDDEN:HIDDEN + STATE_DIM, :] = Wih_p.T
    wks[HIDDEN + STATE_DIM, :] = b_hi
    wks[HIDDEN + STATE_DIM + 1, :] = b_lo

    w1r = np.zeros((128, MLP_HIDDEN), np.float64)
    w1r[0:64, :] = W1.astype(np.float64).T
    w1r[64:128, :] = W1.astype(np.float64).T
    w2r = np.zeros((64, 1), np.float64)
    w2r[0:32, 0] = W2[0].astype(np.float64)
    w2r[32:64, 0] = W2[0].astype(np.float64)
    b1r = np.zeros((64, 1), np.float32)
    b1r[0:32, 0] = b1
    b1r[32:64, 0] = b1
    b2r = np.full((128, 1), b2[0], np.float32)

    # xt arrays: [KX, B] bf16: rows 0:18 = x_t.T, rows 18,19 = ones
    xts = []
    for t in range(SEQ_LEN):
        a = np.ones((KX, B), np.float32)
        a[0:STATE_DIM, :] = state_seq[:, t, :].T
        xts.append(a.astype(bf))

    shared = {
        "wk0": wk0.astype(bf), "wks": wks.astype(bf),
        "w1r": w1r.astype(bf), "w2r": w2r.astype(bf),
        "b1r": b1r, "b2r": b2r,
    }
    in_maps = []
    for cc in range(NCORES):
        lo, hi = cc * BCORE, (cc + 1) * BCORE
        m = dict(shared)
        m["xt0"] = np.ascontiguousarray(xts[0][:, lo:hi])
        m["xt1"] = np.ascontiguousarray(xts[1][:, lo:hi])
        m["xt2"] = np.ascontiguousarray(xts[2][:, lo:hi])
        in_maps.append(m)
    return in_maps


_cached = {}


def kernel(**inputs) -> np.ndarray:
    if "nc" not in _cached:
        _cached["nc"] = build_bass()
    nc = _cached["nc"]
    in_maps = _host_prep(**inputs)
    trace = bool(int(os.environ.get("K_TRACE", "0")))
    res = bass_utils.run_bass_kernel_spmd(nc, in_maps, core_ids=list(range(NCORES)),
                                          trace=trace)
    outs = [r["out"].reshape(-1) for r in res.results]
    _cached["last_results"] = res
    return np.concatenate(outs).astype(np.float32)


# revision 13
# speedup vs baseline: 91.6133x; 91.6133x over previous
"""Trainium2 Bass kernel for nn_CriticNetwork (LSTM T=3, D=18, H=64 + MLP 64->32->1).

Strategy: pure data parallel over 8 NeuronCores (65536 batch each).
Per core, batch is processed in 64 iterations of 1024 elements
(two 512-wide sub-tiles A/B occupying partition rows 0:64 / 64:128 of
every pointwise tile).

Layout: feature-rows on partitions, batch on the free dim.
Gate preacts are produced by single K-stacked bf16 matmuls
  rhs = [h (64) ; x_t (18) ; ones (2: bias hi/lo)]  -> K=84 (step1: K=20)
into PSUM chunks [i|f|o] (3 banks) and [g] (1 bank).
sigmoid/tanh_g run on ScalarE straight from PSUM (bf16 out),
tanh(c) is a custom 8-stage DVE op (odd deg-7 poly, |c|<=1.5),
products run on VectorE in bf16 (2x mode), c = ig+fc on GpSimd.
MLP + final dot use tile-positioned small matmuls.
"""
import os
import numpy as np
import ml_dtypes

import concourse.bacc as bacc
import concourse.bass as bass
import concourse.mybir as mybir
import concourse.tile as tile
from concourse import bass_utils

F32 = mybir.dt.float32
BF16 = mybir.dt.bfloat16
AF = mybir.ActivationFunctionType

NCORES = 8
W = 512                        # sub-tile width (psum bank)
ITERS = int(os.environ.get("K_ITERS", "64"))
BCORE = ITERS * 2 * W          # 65536 at full size
BATCH = BCORE * NCORES
BLK = int(os.environ.get("K_BLK", "8"))  # iters per phase/DMA block
NBLK = ITERS // BLK

STATE_DIM, SEQ_LEN, HIDDEN, MLP_HIDDEN = 18, 3, 64, 32
KX = STATE_DIM + 2             # x rows + two ones rows (bias hi/lo)
KS = HIDDEN + KX               # 84: [h; x; ones]

# tanh deg-7 odd poly on [-1.5, 1.5], maxerr 5.1e-4
T7C = (0.9967175625159229, -0.3102624127429846,
       0.08661915494425512, -0.011767701262857437)

_tanh7c = None


def get_tanh7c():
    global _tanh7c
    if _tanh7c is not None:
        return _tanh7c
    import concourse.dve_ops as dve_ops
    from concourse.dve_spec import (Spec, Src0, C0, C1, C2, C3, sq, lower,
                                    _spill_c3_to_src1)
    from concourse.dve_uop import DveOpSpec

    name = "TANH7C_ANT"
    for op in dve_ops.OPS:
        if op.name == name:
            _tanh7c = op
            return op
    u = sq(Src0)
    body = _spill_c3_to_src1((((C3 * u + C2) * u + C1) * u + C0) * Src0)

    def _ref(in0, in1, s0, s1, imm2):
        uu = in0.astype(np.float32) ** 2
        c3 = np.asarray(in1, np.float32).reshape(in1.shape[0], 1)
        return ((((c3 * uu + imm2) * uu + s1) * uu) + s0) * in0

    spec = Spec(body=body, reference=_ref)
    if name not in dve_ops._SUB_OPCODE_FOR_NAME:
        dve_ops._SUB_OPCODE_FOR_NAME[name] = (
            max(dve_ops._SUB_OPCODE_FOR_NAME.values()) + 1)
    shas = {}
    for ver in ("v3", "v4"):
        try:
            s = DveOpSpec(name=name, opcode=dve_ops.get_dve_sub_opcode(name),
                          uops=lower(spec, ver=ver), rd1_en=True)
            shas[ver] = s.sha(ver)
        except Exception:
            pass
    op = dve_ops.DveOp(name, spec, subdim=False, uops_sha=shas)
    dve_ops.OPS.append(op)
    _tanh7c = op
    return op


# p_ifo free-dim slots (cols) per gate; g goes to its own bank
SLOT = {"i": 0, "f": W, "o": 2 * W}
# weight-matrix column base per gate in the packed wk tiles: [i f o g]
WCOL = {"i": 0, "f": 64, "o": 128, "g": 192}


def build_bass():
    t7 = get_tanh7c()
    nc = bacc.Bacc("TRN2", target_bir_lowering=False, debug=False)

    xt0_d = nc.dram_tensor("xt0", [KX, BCORE], BF16, kind="ExternalInput").ap()
    xt1_d = nc.dram_tensor("xt1", [KX, BCORE], BF16, kind="ExternalInput").ap()
    xt2_d = nc.dram_tensor("xt2", [KX, BCORE], BF16, kind="ExternalInput").ap()
    wk0_d = nc.dram_tensor("wk0", [KX, 256], BF16, kind="ExternalInput").ap()
    wks_d = nc.dram_tensor("wks", [KS, 256], BF16, kind="ExternalInput").ap()
    w1_d = nc.dram_tensor("w1r", [128, MLP_HIDDEN], BF16, kind="ExternalInput").ap()
    w2_d = nc.dram_tensor("w2r", [64, 1], BF16, kind="ExternalInput").ap()
    b1_d = nc.dram_tensor("b1r", [64, 1], F32, kind="ExternalInput").ap()
    b2_d = nc.dram_tensor("b2r", [128, 1], F32, kind="ExternalInput").ap()
    out_d = nc.dram_tensor("out", [2 * ITERS, W], F32, kind="ExternalOutput").ap()

    BW = BLK * 2 * W  # dma block width (4096)

    with tile.TileContext(nc) as tc:
        with tc.tile_pool(name="const", bufs=1) as cpool, \
             tc.tile_pool(name="xt0", bufs=2) as xt0p, \
             tc.tile_pool(name="s1", bufs=2) as s1p, \
             tc.tile_pool(name="s2", bufs=2) as s2p, \
             tc.tile_pool(name="sg", bufs=5) as sgp, \
             tc.tile_pool(name="tg", bufs=4) as tgp, \
             tc.tile_pool(name="work", bufs=5) as wkp, \
             tc.tile_pool(name="cwork", bufs=2 * BLK + 2) as cwp, \
             tc.tile_pool(name="h3", bufs=3) as h3p, \
             tc.tile_pool(name="zr", bufs=3) as zrp, \
             tc.tile_pool(name="vout", bufs=2) as vop, \
             tc.tile_pool(name="pifo", bufs=2, space="PSUM") as pifop, \
             tc.tile_pool(name="pg", bufs=1, space="PSUM") as pgp, \
             tc.tile_pool(name="pv", bufs=1, space="PSUM") as pvp:

            wk0 = cpool.tile([KX, 256], BF16)
            nc.sync.dma_start(wk0[:], wk0_d[:])
            wks = cpool.tile([KS, 256], BF16)
            nc.sync.dma_start(wks[:], wks_d[:])
            w1r = cpool.tile([128, MLP_HIDDEN], BF16)
            nc.sync.dma_start(w1r[:], w1_d[:])
            w2r = cpool.tile([64, 1], BF16)
            nc.sync.dma_start(w2r[:], w2_d[:])
            b1r = cpool.tile([64, 1], F32)
            nc.sync.dma_start(b1r[:], b1_d[:])
            b2t = cpool.tile([128, 1], F32)
            nc.sync.dma_start(b2t[:], b2_d[:])
            c3t = cpool.tile([128, 1], F32)
            nc.vector.memset(c3t[:], T7C[3])

            def gate_mms(wk, kk, rhs_blk, cA, p_ifo, p_g, gates):
                """Emit A/B col-paired matmuls for `gates` of one step.
                rhs_blk: block tile whose rows 0:kk are the stacked rhs;
                cA: column offset of sub-tile A inside the block."""
                for gn in gates:
                    lhs = wk[0:kk, WCOL[gn]:WCOL[gn] + 64]
                    if gn == "g":
                        oA, oB = p_g[0:64, :], p_g[64:128, :]
                    else:
                        s = SLOT[gn]
                        oA, oB = p_ifo[0:64, s:s + W], p_ifo[64:128, s:s + W]
                    nc.tensor.matmul(oA, lhs, rhs_blk[0:kk, cA:cA + W],
                                     start=True, stop=True, tile_position=(0, 0),
                                     skip_group_check=True)
                    nc.tensor.matmul(oB, lhs, rhs_blk[0:kk, cA + W:cA + 2 * W],
                                     start=True, stop=True, tile_position=(0, 64),
                                     skip_group_check=True)

            vb = None
            for blk in range(NBLK):
                xt0b = xt0p.tile([KX, BW], BF16)
                nc.sync.dma_start(xt0b[:], xt0_d[:, blk * BW:(blk + 1) * BW])
                s1b = s1p.tile([KS, BW], BF16)
                nc.sync.dma_start(s1b[HIDDEN:KS, :], xt1_d[:, blk * BW:(blk + 1) * BW])
                s2b = s2p.tile([KS, BW], BF16)
                nc.sync.dma_start(s2b[HIDDEN:KS, :], xt2_d[:, blk * BW:(blk + 1) * BW])

                # ------- phase 1: step 1 for all BLK iters (h0 = 0) -----------
                cs = {}
                for j in range(BLK):
                    cA = j * 2 * W
                    p_ifo = pifop.tile([128, 3 * W], F32, tag="pifo")
                    p_g = pgp.tile([128, W], F32, tag="pg")
                    gate_mms(wk0, KX, xt0b, cA, p_ifo, p_g, ("i", "o", "g"))

                    sg1 = sgp.tile([128, 3 * W], BF16, tag="sg")
                    nc.scalar.activation(sg1[:, 0:W], p_ifo[:, 0:W], AF.Sigmoid)
                    nc.scalar.activation(sg1[:, 2 * W:3 * W], p_ifo[:, 2 * W:3 * W],
                                         AF.Sigmoid)
                    tg1 = tgp.tile([128, W], BF16, tag="tg")
                    nc.scalar.activation(tg1[:], p_g[:], AF.Tanh)

                    c1 = cwp.tile([128, W], BF16, tag="c")
                    nc.vector.tensor_mul(c1[:], sg1[:, 0:W], tg1[:])
                    th1 = wkp.tile([128, W], BF16, tag="th")
                    nc.vector._custom_dve(t7, out=th1[:], in0=c1[:], in1=c3t[:],
                                          s0=T7C[0], s1=T7C[1], imm2=T7C[2])
                    # h1 -> s1 block rows 0:64 (A and B halves)
                    nc.gpsimd.tensor_mul(s1b[0:HIDDEN, cA:cA + W],
                                          sg1[0:64, 2 * W:3 * W], th1[0:64, :])
                    nc.gpsimd.tensor_mul(s1b[0:HIDDEN, cA + W:cA + 2 * W],
                                          sg1[64:128, 2 * W:3 * W], th1[64:128, :])
                    cs[j] = c1

                # ------- phase 2: step 2 for all BLK iters --------------------
                for j in range(BLK):
                    cA = j * 2 * W
                    p_ifo2 = pifop.tile([128, 3 * W], F32, tag="pifo")
                    p_g2 = pgp.tile([128, W], F32, tag="pg")
                    gate_mms(wks, KS, s1b, cA, p_ifo2, p_g2, ("i", "f", "o", "g"))

                    sg2 = sgp.tile([128, 3 * W], BF16, tag="sg")
                    nc.scalar.activation(sg2[:], p_ifo2[:], AF.Sigmoid)
                    tg2 = tgp.tile([128, W], BF16, tag="tg")
                    nc.scalar.activation(tg2[:], p_g2[:], AF.Tanh)

                    ig2 = wkp.tile([128, W], BF16, tag="ig")
                    nc.vector.tensor_mul(ig2[:], sg2[:, 0:W], tg2[:])
                    fc2 = wkp.tile([128, W], BF16, tag="fc")
                    nc.vector.tensor_mul(fc2[:], sg2[:, W:2 * W], cs[j][:])
                    c2 = cwp.tile([128, W], BF16, tag="c")
                    nc.vector.tensor_add(c2[:], ig2[:], fc2[:])
                    th2 = wkp.tile([128, W], BF16, tag="th")
                    nc.vector._custom_dve(t7, out=th2[:], in0=c2[:], in1=c3t[:],
                                          s0=T7C[0], s1=T7C[1], imm2=T7C[2])
                    nc.gpsimd.tensor_mul(s2b[0:HIDDEN, cA:cA + W],
                                          sg2[0:64, 2 * W:3 * W], th2[0:64, :])
                    nc.gpsimd.tensor_mul(s2b[0:HIDDEN, cA + W:cA + 2 * W],
                                          sg2[64:128, 2 * W:3 * W], th2[64:128, :])
                    cs[j] = c2

                # ------- phase 3: step 3 + MLP + value ------------------------
                for j in range(BLK):
                    it = blk * BLK + j
                    cA = j * 2 * W
                    p_ifo3 = pifop.tile([128, 3 * W], F32, tag="pifo")
                    p_g3 = pgp.tile([128, W], F32, tag="pg")
                    gate_mms(wks, KS, s2b, cA, p_ifo3, p_g3, ("i", "f", "o", "g"))

                    sg3 = sgp.tile([128, 3 * W], BF16, tag="sg")
                    nc.scalar.activation(sg3[:], p_ifo3[:], AF.Sigmoid)
                    tg3 = tgp.tile([128, W], BF16, tag="tg")
                    nc.scalar.activation(tg3[:], p_g3[:], AF.Tanh)

                    ig3 = wkp.tile([128, W], BF16, tag="ig")
                    nc.vector.tensor_mul(ig3[:], sg3[:, 0:W], tg3[:])
                    fc3 = wkp.tile([128, W], BF16, tag="fc")
                    nc.vector.tensor_mul(fc3[:], sg3[:, W:2 * W], cs[j][:])
                    c3_ = cwp.tile([128, W], BF16, tag="c")
                    nc.vector.tensor_add(c3_[:], ig3[:], fc3[:])
                    th3 = wkp.tile([128, W], BF16, tag="th")
                    nc.vector._custom_dve(t7, out=th3[:], in0=c3_[:], in1=c3t[:],
                                          s0=T7C[0], s1=T7C[1], imm2=T7C[2])
                    h3 = h3p.tile([128, W], BF16, tag="h3")
                    nc.vector.tensor_mul(h3[:], sg3[:, 2 * W:3 * W], th3[:])

                    zp = pifop.tile([64, W], F32, tag="pifo")
                    nc.tensor.matmul(zp[0:32, :], w1r[0:64, :], h3[0:64, :],
                                     start=True, stop=True, tile_position=(0, 0),
                                     skip_group_check=True)
                    nc.tensor.matmul(zp[32:64, :], w1r[64:128, :], h3[64:128, :],
                                     start=True, stop=True, tile_position=(64, 32),
                                     skip_group_check=True)
                    zr = zrp.tile([64, W], BF16, tag="zr")
                    nc.scalar.activation(zr[:], zp[:], AF.Relu, bias=b1r[:])

                    if it % 2 == 0:
                        vb = pvp.tile([97, W], F32, tag="vb")
                    r = (it % 2) * 64
                    nc.tensor.matmul(vb[r:r + 1, :], w2r[0:32, :], zr[0:32, :],
                                     start=True, stop=True, tile_position=(0, r),
                                     skip_group_check=True)
                    nc.tensor.matmul(vb[r + 32:r + 33, :], w2r[32:64, :],
                                     zr[32:64, :], start=True, stop=True,
                                     tile_position=(32, r + 32),
                                     skip_group_check=True)

                    if it % 2 == 1:
                        vo = vop.tile([97, W], F32, tag="vo")
                        nc.scalar.activation(vo[:], vb[:], AF.Identity,
                                             bias=b2t[0:97, :])
                        e = it - 1
                        for s, row in ((2 * e, 0), (2 * e + 1, 32),
                                       (2 * e + 2, 64), (2 * e + 3, 96)):
                            nc.sync.dma_start(out_d[s:s + 1, :], vo[row:row + 1, :])

    nc.compile()
    return nc


def _host_prep(state_seq, W_ih, W_hh, b_ih, b_hh, W1, b1, W2, b2):
    """Build per-core input maps (host-side layout prep only)."""
    bf = ml_dtypes.bfloat16
    B = state_seq.shape[0]
    b = (b_ih.astype(np.float64) + b_hh.astype(np.float64))  # [256]

    # gate reorder [i f o g] columns
    perm = np.concatenate([np.arange(0, 64), np.arange(64, 128),
                           np.arange(192, 256), np.arange(128, 192)])
    # packed weight cols: wk[:, c] for output gate-row perm[c]
    Wih_p = W_ih[perm, :].astype(np.float64)   # [256, 18]
    Whh_p = W_hh[perm, :].astype(np.float64)   # [256, 64]
    b_p = b[perm]                               # [256]

    b_hi = b_p.astype(bf).astype(np.float64)
    b_lo = (b_p - b_hi).astype(bf).astype(np.float64)

    wk0 = np.zeros((KX, 256), np.float64)
    wk0[0:STATE_DIM, :] = Wih_p.T
    wk0[STATE_DIM, :] = b_hi
    wk0[STATE_DIM + 1, :] = b_lo
    wks = np.zeros((KS, 256), np.float64)
    wks[0:HIDDEN, :] = Whh_p.T
    wks[HIDDEN:HIDDEN + STATE_DIM, :] = Wih_p.T
    wks[HIDDEN + STATE_DIM, :] = b_hi
    wks[HIDDEN + STATE_DIM + 1, :] = b_lo

    w1r = np.zeros((128, MLP_HIDDEN), np.float64)
    w1r[0:64, :] = W1.astype(np.float64).T
    w1r[64:128, :] = W1.astype(np.float64).T
    w2r = np.zeros((64, 1), np.float64)
    w2r[0:32, 0] = W2[0].astype(np.float64)
    w2r[32:64, 0] = W2[0].astype(np.float64)
    b1r = np.zeros((64, 1), np.float32)
    b1r[0:32, 0] = b1
    b1r[32:64, 0] = b1
    b2r = np.full((128, 1), b2[0], np.float32)

    # xt arrays: [KX, B] bf16: rows 0:18 = x_t.T, rows 18,19 = ones
    xts = []
    for t in range(SEQ_LEN):
        a = np.ones((KX, B), np.float32)
        a[0:STATE_DIM, :] = state_seq[:, t, :].T
        xts.append(a.astype(bf))

    shared = {
        "wk0": wk0.astype(bf), "wks": wks.astype(bf),
        "w1r": w1r.astype(bf), "w2r": w2r.astype(bf),
        "b1r": b1r, "b2r": b2r,
    }
    in_maps = []
    for cc in range(NCORES):
        lo, hi = cc * BCORE, (cc + 1) * BCORE
        m = dict(shared)
        m["xt0"] = np.ascontiguousarray(xts[0][:, lo:hi])
        m["xt1"] = np.ascontiguousarray(xts[1][:, lo:hi])
        m["xt2"] = np.ascontiguousarray(xts[2][:, lo:hi])
        in_maps.append(m)
    return in_maps


_cached = {}


def kernel(**inputs) -> np.ndarray:
    if "nc" not in _cached:
        _cached["nc"] = build_bass()
    nc = _cached["nc"]
    in_maps = _host_prep(**inputs)
    trace = bool(int(os.environ.get("K_TRACE", "0")))
    res = bass_utils.run_bass_kernel_spmd(nc, in_maps, core_ids=list(range(NCORES)),
                                          trace=trace)
    outs = [r["out"].reshape(-1) for r in res.results]
    _cached["last_results"] = res
    return np.concatenate(outs).astype(np.float32)
